# revision 1
# baseline (speedup 1.0000x reference)
"""Trainium2 Bass kernel for nn_MA_73478300500338 (retrieval_knn).

Pipeline (reference semantics):
  q = relu(query_embedding)                      [B, D]
  sim = cos(q, memory_keys); idx = top_k(sim, 32)
  mk = memory_keys[idx]
  qt = relu(q @ Wq + bq); mt = relu(mk @ Wm + bm)
  attended = sum_j mt[:, j, :]   (softmax over size-1 axis == 1)
  ma = LN(attended + qt) * gamma + beta
  out = [q, ma] @ Wc + bc                        [B, C]

Distribution (8 NeuronCores):
  Phase 1 (candidate scan): memory bank sharded 8x (12500 rows/core, padded
    to 13312). Keys are L2-normalized on host (ranking-invariant), scaled and
    cast to fp8e4m3. Each core computes all 256 queries x 13312 dots with
    fp8 DoubleRow matmuls (2 contraction rows/partition, 0.5 cyc/row), then
    selects candidates: Act copies PSUM->SBUF fp16, DVE does 3 contiguous
    pairwise-max rounds (group size 8) + Max8/MaxIndex per unit (units of
    1024/2048 keys). That yields top-8 groups-of-8 per unit.
  Host: merges 64 x 8 candidate groups/query, exactly rescores the members
    of the TOPG best groups in fp32 (and any unit that reported a
    duplicated index near the cut, to be robust to fp16 value ties and fp8
    dot noise), picks the exact top-32. Host work is pure indexing plus a
    small (256 x ~1024 x 512) batched dot product.
  Phase 2 (attention MLP): queries sharded 8x (32/core). mtT = relu(Wm^T mk
    + bm) runs in bf16 with au on partitions (per-partition bias), the sum
    over the 32 retrieved keys runs on DVE tensor_reduce, qt and the output
    projection run in fp32 (no separate Ldweights), and the entire
    layernorm affine (gamma, beta, mean, rstd) is folded into the output
    projection: out = q @ Wc_q + bc' + rstd*(x @ (gamma*Wc_ma) - mu*S).
"""

import os
import sys
import json

import numpy as np
import ml_dtypes

os.environ.setdefault("MYCRO_LOCAL_CACHE", "1")
if "/opt/trn_rl_repo" not in sys.path:
    sys.path.insert(0, "/opt/trn_rl_repo")

try:
    import jax as _jax
    _jax.config.update("jax_compilation_cache_dir", "/tmp/jax_cache_nn_ma")
    _jax.config.update("jax_persistent_cache_min_entry_size_bytes", -1)
    _jax.config.update("jax_persistent_cache_min_compile_time_secs", 0.5)
except Exception:
    pass

import bass_rust
import concourse.bass as bass
import concourse.bacc as bacc
import concourse.mybir as mybir
import concourse.tile as tile
from concourse.vector_clock import ScopedClock

# ---------------------------------------------------------------------------
# Workaround: this walrus build supports a single sync-wait per CTRL
# instruction, but Tile's stock tail drain carries one wait per busy
# processor. Split them into standalone single-wait instructions.
# ---------------------------------------------------------------------------


def _patched_drain_and_barrier(self, tick_clock, wait_clock):
    nc = self.nc
    with nc.discard():
        probe = nc.sync.drain()
        wait_clock.add_sem_waits(
            probe.ins, ScopedClock({None: tick_clock.global_clock})
        )
        j = json.loads(nc.instruction_to_json(probe.ins))
    waits = (j.get("sync_info") or {}).get("on_wait") or []
    for w in waits:
        sem = bass_rust.SemaphoreHandle(w["ant_name"], w["id"])
        assert w["wait_mode"] == "sem-ge-imm", w
        nc.sync.wait_ge(sem, w["wait_value"])
    nc.sync.drain()
    nc.all_engine_barrier()
    popped = nc._tile_sem_poison_stack.pop()
    assert popped is self._sem_poison
    nc.clear_and_free_semaphores(list(self.sems.allocated().values()))
    nc.all_engine_barrier()


tile.TileContext._drain_and_barrier = _patched_drain_and_barrier

# ---------------------------------------------------------------------------
# Problem shapes (hardcoded per spec)
# ---------------------------------------------------------------------------
B, N, D = 256, 100000, 512
AU, C, K = 256, 100, 32
NCORES = 8
SH = N // NCORES            # 12500 keys per core
SHP = 13312                 # padded shard width (13 x 1024, %16 == 0)
UNIT = 2048                 # selection unit width (keys)
NU = 9                      # units: 1024x5, 2048x4
G = 8                       # group size (keys per candidate group)
TOPG = 128                  # groups rescored exactly per query
KSCALE = 64.0               # fp8 key scale (ranking-invariant)
EPS_LN = 1e-5

F32 = mybir.dt.float32
F16 = mybir.dt.float16
BF16 = mybir.dt.bfloat16
FP8 = mybir.dt.float8e4
U16 = mybir.dt.uint16
F8NP = ml_dtypes.float8_e4m3
BF16NP = ml_dtypes.bfloat16


def _ubase(u):
    """Unit boundaries: 5x1024 ramp-in then 4x2048 (sums to 13312)."""
    return min(u * 1024 if u <= 5 else 5120 + (u - 5) * 2048, SHP)


_cache = {}


# ---------------------------------------------------------------------------
# Phase 1: fp8 DoubleRow dots + grouped top-8 candidates per unit
# ---------------------------------------------------------------------------


def _build_phase1():
    nc = bacc.Bacc()
    q8d = nc.dram_tensor("q8", [128, 2, 2, B], FP8, kind="ExternalInput")
    k8d = nc.dram_tensor("k8", [128, 2, 2, SHP], FP8, kind="ExternalInput")
    t8 = nc.dram_tensor("t8", [2, 128, NU, 8], F16, kind="ExternalOutput")
    i8 = nc.dram_tensor("i8", [2, 128, NU, 8], U16, kind="ExternalOutput")

    with tile.TileContext(nc) as tc:
        with (
            tc.tile_pool(name="persist", bufs=1) as persist,
            tc.tile_pool(name="work", bufs=4) as wp,
            tc.tile_pool(name="psum", bufs=2, space="PSUM") as psump,
        ):
            # Copy-table preload (overlaps the initial DMA wait)
            dum = wp.tile([1, 2], F32, tag="dum")
            nc.vector.memset(dum, 0.0)
            nc.scalar.copy(out=dum[:, 1:2], in_=dum[:, 0:1])

            q8 = persist.tile([128, 2, 2, B], FP8)
            nc.sync.dma_start(out=q8, in_=q8d[:, :, :, :])
            k8 = persist.tile([128, 2, 2, SHP], FP8)
            # unit 0 is small (1024) and arrives as 2 fine DMAs so the
            # copy/select pipeline starts as early as possible
            for s in range(2):
                nc.sync.dma_start(out=k8[:, :, :, s * 512:(s + 1) * 512],
                                  in_=k8d[:, :, :, s * 512:(s + 1) * 512])
            # units 1-2 land as fine 512-col DMAs to close the early Act gap
            for s in range(2, 12):
                nc.sync.dma_start(out=k8[:, :, :, s * 512:(s + 1) * 512],
                                  in_=k8d[:, :, :, s * 512:(s + 1) * 512])
            for s in range(6, SHP // 1024):
                lo, hi = s * 1024, (s + 1) * 1024
                nc.sync.dma_start(out=k8[:, :, :, lo:hi], in_=k8d[:, :, :, lo:hi])

            t8s = persist.tile([128, 2, NU, 8], F16)
            i8s = persist.tile([128, 2, NU, 8], U16)

            for u in range(NU):
                lo, hi = _ubase(u), _ubase(u + 1)
                uw = hi - lo                       # 1024 for unit 0, else 2048
                ng = uw // G                       # groups in this unit
                for bc in range(2):
                    ps = psump.tile([128, UNIT], F32, tag="ps")
                    for w2 in range(uw // 512):
                        for kc in range(2):
                            nc.tensor.matmul(
                                ps[:, w2 * 512:(w2 + 1) * 512],
                                q8[:, kc, :, bc * 128:(bc + 1) * 128],
                                k8[:, kc, :, lo + w2 * 512:lo + (w2 + 1) * 512],
                                start=(kc == 0), stop=(kc == 1),
                                perf_mode=mybir.MatmulPerfMode.DoubleRow,
                            )
                    dw = wp.tile([128, UNIT], F16, tag="dw")
                    nc.scalar.copy(out=dw[:, :uw], in_=ps[:, :uw])
                    h = uw // 2
                    t1 = wp.tile([128, UNIT // 2], F16, tag="t1")
                    nc.vector.tensor_max(out=t1[:, :h], in0=dw[:, :h], in1=dw[:, h:uw])
                    t2 = wp.tile([128, UNIT // 4], F16, tag="t2")
                    nc.vector.tensor_max(out=t2[:, :h // 2], in0=t1[:, :h // 2],
                                         in1=t1[:, h // 2:h])
                    g8 = wp.tile([128, UNIT // 8], F16, tag="g8")
                    nc.vector.tensor_max(out=g8[:, :ng], in0=t2[:, :ng],
                                         in1=t2[:, ng:2 * ng])
                    nc.vector.max(out=t8s[:, bc, u, :], in_=g8[:, :ng])
                    nc.vector.max_index(out=i8s[:, bc, u, :],
                                        in_max=t8s[:, bc, u, :], in_values=g8[:, :ng])

            nus = NU * 8
            nc.sync.dma_start(
                out=bass.AP(t8, 0, [[nus, 128], [128 * nus, 2], [1, nus]]),
                in_=t8s.rearrange("p b u s -> p b (u s)"))
            nc.sync.dma_start(
                out=bass.AP(i8, 0, [[nus, 128], [128 * nus, 2], [1, nus]]),
                in_=i8s.rearrange("p b u s -> p b (u s)"))
    nc.finalize()
    return nc


# ---------------------------------------------------------------------------
# Phase 2: attention MLP + LN + output projection (32 queries per core, bf16)
# ---------------------------------------------------------------------------
BQ = B // NCORES            # 32 queries per core
NK = BQ * K                 # 1024 gathered key columns per core
DC5 = 5                     # 4 d-chunks + 1 bias-aug chunk


BLOB_WQ = 0                 # fp32 blob layout (columns)
BLOB_WC = 1024              # 7 chunks x 100
BLOB_ID = 1724              # identity 128
BLOB_W = 1852


def _build_phase2():
    nc = bacc.Bacc()
    cst_ = nc.dram_tensor("cst", [128, 4], F32, kind="ExternalInput")
    wmk_ = nc.dram_tensor("wmk", [128, 4, AU + NK], BF16, kind="ExternalInput")
    qta_ = nc.dram_tensor("qta", [128, DC5, BQ], F32, kind="ExternalInput")
    blob_ = nc.dram_tensor("blob", [128, BLOB_W], F32, kind="ExternalInput")
    srow_ = nc.dram_tensor("srow", [C], F32, kind="ExternalInput")
    out = nc.dram_tensor("out", [BQ, C], F32, kind="ExternalOutput")

    RELU = mybir.ActivationFunctionType.Relu
    with tile.TileContext(nc) as tc:
        with (
            tc.tile_pool(name="p", bufs=1) as pool,
            tc.tile_pool(name="psmt", bufs=3, space="PSUM") as psmt,
            tc.tile_pool(name="psq", bufs=1, space="PSUM") as psq,
            tc.tile_pool(name="ps1", bufs=1, space="PSUM") as ps1,
        ):
            # activation-table preloads (overlap the DMA wait)
            dum = pool.tile([1, 2], F32)
            nc.vector.memset(dum, 0.0)
            nc.scalar.activation(out=dum[:, 1:2], in_=dum[:, 0:1], func=RELU)
            nc.scalar.activation(out=dum[:, 0:1], in_=dum[:, 1:2],
                                 func=mybir.ActivationFunctionType.Sqrt)

            # PE p-state warm-up: keep the array busy through the DMA
            # preamble so the real matmuls run at peak clock
            wrm = pool.tile([128, 256], BF16)
            nc.vector.memset(wrm, 0.0)
            for w_ in range(24):
                pw = psq.tile([128, BQ], F32, tag="pqt")
                nc.tensor.matmul(pw[:1, :], wrm[:, :1], wrm[:, 128:128 + BQ],
                                 start=True, stop=True)

            # loads: Wm and mk arrive per contraction chunk so the matmul
            # stream starts after the first quarter
            wmk = pool.tile([128, 4, AU + NK], BF16)
            for c in range(4):
                nc.sync.dma_start(out=wmk[:, c, :], in_=wmk_[:, c, :])
            wm = wmk[:, :, :AU]
            mkt = wmk[:, :, AU:]
            cst = pool.tile([128, 4], F32)
            nc.sync.dma_start(out=cst, in_=cst_[:, :])
            qta = pool.tile([128, DC5, BQ], F32)
            nc.sync.dma_start(out=qta, in_=qta_[:, :, :])
            blob = pool.tile([128, BLOB_W], F32)
            nc.sync.dma_start(out=blob, in_=blob_[:, :])
            srt = pool.tile([BQ, C], F32)
            nc.sync.dma_start(out=srt, in_=bass.AP(srow_, 0, [[0, BQ], [1, C]]))

            # mtT[au, nk] = relu(Wm^T mk + bm); bias is per-partition here.
            # Matmuls are emitted chunk-major so the in-order PE queue tracks
            # the chunk DMA arrivals instead of serializing on the last one.
            mtT = pool.tile([128, 2, NK], BF16)
            attT = pool.tile([128, 2, BQ], BF16)
            groups = [(a, h) for a in range(2) for h in range(2)]
            pmts = []
            for _gi in range(len(groups)):
                pmt = psmt.tile([128, NK // 2], F32, tag="pmt")
                pmts.append(pmt)
            for c in range(4):
                for gi, (a, h) in enumerate(groups):
                    nc.tensor.matmul(
                        pmts[gi], wm[:, c, a * 128:(a + 1) * 128],
                        mkt[:, c, h * 512:(h + 1) * 512],
                        start=(c == 0), stop=(c == 3))
            for gi, (a, h) in enumerate(groups):
                nc.scalar.activation(
                    out=mtT[:, a, h * 512:(h + 1) * 512], in_=pmts[gi],
                    func=RELU, bias=cst[:, 2 + a:3 + a], scale=1.0)
                # attT[au, b] = sum_j mtT[au, (b j)] on DVE (fp32 internal)
                with nc.allow_low_precision(
                        reason="DVE reduces in fp32 internally; bf16 "
                               "output rounding is ~0.4% on 2e-2 tol"):
                    nc.vector.tensor_reduce(
                        out=attT[:, a, h * 16:(h + 1) * 16],
                        in_=mtT[:, a, h * 512:(h + 1) * 512].rearrange(
                            "p (b j) -> p b j", j=K),
                        axis=mybir.AxisListType.X, op=mybir.AluOpType.add)

            # qtT[au, b] = relu(Wq^T q + bq), fp32 matmuls (no ldweights)
            xT = pool.tile([128, 2, BQ], F32)
            for a in range(2):
                pqt = psq.tile([128, BQ], F32, tag="pqt")
                for c in range(4):
                    nc.tensor.matmul(
                        pqt, blob[:, BLOB_WQ + c * AU + a * 128:
                                  BLOB_WQ + c * AU + (a + 1) * 128],
                        qta[:, c, :], start=(c == 0), stop=(c == 3))
                qts = pool.tile([128, BQ], F32, tag=f"qts{a}")
                nc.scalar.activation(
                    out=qts, in_=pqt, func=RELU,
                    bias=cst[:, a:a + 1], scale=1.0)
                nc.vector.tensor_add(out=xT[:, a, :], in0=attT[:, a, :],
                                     in1=qts)

            # transpose xT -> x [b, au] (fp32)
            idt = blob[:, BLOB_ID:BLOB_ID + 128]
            x = pool.tile([BQ, AU], F32)
            st6 = pool.tile([BQ, 2, 6], F32)
            for a in range(2):
                pst = ps1.tile([BQ, 128], F32, tag="pst")
                nc.tensor.transpose(pst, xT[:, a, :], idt[:128, :128])
                nc.scalar.copy(out=x[:, a * 128:(a + 1) * 128], in_=pst)
                # layernorm stats per half, overlapping the other transpose;
                # gamma/beta and the (x-mu)*rstd affine are folded into Wc
                nc.vector.bn_stats(out=st6[:, a, :],
                                   in_=x[:, a * 128:(a + 1) * 128])
            st2 = pool.tile([BQ, 2], F32)
            nc.vector.bn_aggr(out=st2, in_=st6)
            eps = pool.tile([BQ, 1], F32)
            nc.vector.memset(eps, EPS_LN)
            sd = pool.tile([BQ, 1], F32)
            nc.scalar.activation(out=sd, in_=st2[:, 1:2],
                                 func=mybir.ActivationFunctionType.Sqrt,
                                 bias=eps, scale=1.0)
            rstd = pool.tile([BQ, 1], F32)
            nc.vector.reciprocal(out=rstd, in_=sd)

            # transpose raw x -> maT [au, b] (fp32); LN is applied via the
            # folded output projection below
            maT = pool.tile([128, 2, BQ], F32)
            for a in range(2):
                pst2 = ps1.tile([128, BQ], F32, tag="pst2")
                nc.tensor.transpose(pst2, x[:, a * 128:(a + 1) * 128],
                                    idt[:BQ, :BQ])
                nc.scalar.copy(out=maT[:, a, :], in_=pst2)

            # out = q @ Wc_q + bc' + rstd*(x @ Wc_ma' - mu*S)
            poq = ps1.tile([BQ, C], F32, tag="poq")
            for c in range(DC5):
                nc.tensor.matmul(
                    poq, qta[:, c, :],
                    blob[:, BLOB_WC + c * C:BLOB_WC + (c + 1) * C],
                    start=(c == 0), stop=(c == DC5 - 1))
            pom = ps1.tile([BQ, C], F32, tag="pom")
            for a in range(2):
                nc.tensor.matmul(
                    pom, maT[:, a, :],
                    blob[:, BLOB_WC + (5 + a) * C:BLOB_WC + (6 + a) * C],
                    start=(a == 0), stop=(a == 1))
            # out = poq + rstd*(pom - mu*S): f = mu*S runs early (only needs
            # mu), the rstd dependency lands in the final fused op
            f1 = pool.tile([BQ, C], F32)
            nc.vector.tensor_scalar(out=f1, in0=srt, scalar1=st2[:, 0:1],
                                    scalar2=None, op0=mybir.AluOpType.mult)
            e2 = pool.tile([BQ, C], F32)
            nc.vector.tensor_sub(out=e2, in0=pom, in1=f1)
            ot = pool.tile([BQ, C], F32)
            nc.vector.scalar_tensor_tensor(out=ot, in0=e2, scalar=rstd,
                                           in1=poq, op0=mybir.AluOpType.mult,
                                           op1=mybir.AluOpType.add)
            nc.sync.dma_start(out=out[:, :], in_=ot)
    nc.finalize()
    return nc


# ---------------------------------------------------------------------------
# SPMD runner with a persistent jitted executable
# ---------------------------------------------------------------------------


class _SpmdRunner:
    def __init__(self, nc, n_cores=NCORES):
        import jax
        from jax.sharding import Mesh, PartitionSpec
        from concourse.bass2jax import (
            _bass_exec_p,
            install_neuronx_cc_hook,
            partition_id_tensor,
        )

        try:
            from jax.experimental.shard_map import shard_map
        except ImportError:
            from jax.shard_map import shard_map

        install_neuronx_cc_hook()
        self.jax = jax
        partition_name = (
            nc.partition_id_tensor.name if nc.partition_id_tensor else None
        )
        in_names, out_names, out_avals, zero_outs = [], [], [], []
        for alloc in nc.m.functions[0].allocations:
            if not isinstance(alloc, mybir.MemoryLocationSet):
                continue
            name = alloc.memorylocations[0].name
            if alloc.kind == "ExternalInput":
                if name != partition_name:
                    in_names.append(name)
            elif alloc.kind == "ExternalOutput":
                shape = tuple(alloc.tensor_shape)
                dtype = mybir.dt.np(alloc.dtype)
                out_names.append(name)
                out_avals.append(jax.core.ShapedArray(shape, dtype))
                zero_outs.append(np.zeros((n_cores * shape[0], *shape[1:]), dtype))
        self.in_names = list(in_names)
        self.out_names = out_names
        self.out_avals = out_avals
        self.zero_outs = zero_outs
        self.n_cores = n_cores
        n_params = len(in_names)
        n_outs = len(out_names)
        all_in = in_names + out_names + ([partition_name] if partition_name else [])

        def _body(*args):
            operands = list(args)
            if partition_name is not None:
                operands.append(partition_id_tensor())
            return tuple(
                _bass_exec_p.bind(
                    *operands,
                    out_avals=tuple(out_avals),
                    in_names=tuple(all_in),
                    out_names=tuple(out_names),
                    lowering_input_output_aliases=(),
                    sim_require_finite=True,
                    sim_require_nnan=True,
                    nc=nc,
                )
            )

        devices = jax.devices()[:n_cores]
        mesh = Mesh(np.asarray(devices), ("core",))
        in_specs = (PartitionSpec("core"),) * (n_params + n_outs)
        out_specs = (PartitionSpec("core"),) * n_outs
        self.sharded = jax.jit(
            shard_map(
                _body, mesh=mesh, in_specs=in_specs, out_specs=out_specs,
                check_rep=False,
            ),
            donate_argnums=tuple(range(n_params, n_params + n_outs)),
            keep_unused=True,
        )

    def __call__(self, concat_in):
        args = [concat_in[n] for n in self.in_names]
        zeros = [np.zeros_like(z) for z in self.zero_outs]
        out_arrs = self.sharded(*args, *zeros)
        res = []
        for c in range(self.n_cores):
            res.append({
                name: np.asarray(out_arrs[i]).reshape(
                    self.n_cores, *self.out_avals[i].shape
                )[c]
                for i, name in enumerate(self.out_names)
            })
        return res


def _rep(a):
    a = np.ascontiguousarray(a)
    return np.broadcast_to(a, (NCORES,) + a.shape).reshape(
        NCORES * a.shape[0], *a.shape[1:]
    )


# ---------------------------------------------------------------------------
# Host orchestration
# ---------------------------------------------------------------------------


def kernel(**inputs):
    qe = np.asarray(inputs["query_embedding"], dtype=np.float32)
    keys = np.asarray(inputs["memory_keys"], dtype=np.float32)
    Wq = np.asarray(inputs["Wq"], dtype=np.float32)
    bq = np.asarray(inputs["bq"], dtype=np.float32)
    Wm = np.asarray(inputs["Wm"], dtype=np.float32)
    bm = np.asarray(inputs["bm"], dtype=np.float32)
    gam = np.asarray(inputs["ln_gamma"], dtype=np.float32)
    bet = np.asarray(inputs["ln_beta"], dtype=np.float32)
    Wc = np.asarray(inputs["Wc"], dtype=np.float32)
    bc_ = np.asarray(inputs["bc"], dtype=np.float32)
    k = int(inputs["k"])
    assert k == K and qe.shape == (B, D) and keys.shape == (N, D)

    # ---- host prep: normalize keys, fp8 layouts ----
    mn = np.sqrt(np.einsum("nd,nd->n", keys, keys, dtype=np.float64)).astype(np.float32)
    kn = keys * (1.0 / mn)[:, None]                 # [N, D] fp32, for rescoring
    qr_full = np.maximum(qe, 0.0)                   # [B, D] fp32 relu'd queries

    k8 = (kn * KSCALE).astype(F8NP)                 # [N, D] fp8
    q8T = np.ascontiguousarray(qr_full.T).astype(F8NP)   # [D, B] fp8
    q8_dev = q8T.reshape(2, 2, 128, B).transpose(2, 0, 1, 3)  # [128,2,2,B]

    import jax
    from jax.sharding import Mesh, NamedSharding, PartitionSpec
    devices = jax.devices()[:NCORES]
    mesh = Mesh(np.asarray(devices), ("core",))
    csh = NamedSharding(mesh, PartitionSpec("core"))

    parts = []
    for c in range(NCORES):
        kT = np.zeros((D, SHP), F8NP)
        kT[:, :SH] = k8[c * SH:(c + 1) * SH].T
        shard = np.ascontiguousarray(
            kT.reshape(2, 2, 128, SHP).transpose(2, 0, 1, 3))
        parts.append(jax.device_put(shard, devices[c]))
    k8_dev = jax.make_array_from_single_device_arrays(
        (NCORES * 128, 2, 2, SHP), csh, parts)

    if "r1" not in _cache:
        _cache["r1"] = _SpmdRunner(_build_phase1())
    res1 = _cache["r1"]({"q8": _rep(np.ascontiguousarray(q8_dev)), "k8": k8_dev})

    # ---- host merge: decode candidates, exact rescore, top-32 ----
    NG = NU * 8                                     # 56 candidate groups/core
    vals = np.empty((B, NCORES, NG), np.float32)
    gidx = np.empty((B, NCORES, NG), np.int64)      # group code: u*256 + g
    ucode = (np.arange(NU, dtype=np.int64) * 256).repeat(8)[None, :]
    for c in range(NCORES):
        t = res1[c]["t8"].reshape(2 * 128, NG).astype(np.float32)
        i = res1[c]["i8"].reshape(2 * 128, NG).astype(np.int64)
        vals[:, c, :] = t
        gidx[:, c, :] = i + ucode

    fvals = vals.reshape(B, NCORES * NG)
    fcore = np.broadcast_to(np.arange(NCORES)[None, :, None],
                            (B, NCORES, NG)).reshape(B, NCORES * NG)
    fgidx = gidx.reshape(B, NCORES * NG)

    top = np.argpartition(-fvals, TOPG - 1, axis=1)[:, :TOPG]   # [B, TOPG]
    tcore = np.take_along_axis(fcore, top, axis=1)
    tg = np.take_along_axis(fgidx, top, axis=1)
    tu, tgg = tg >> 8, tg & 255
    # member columns within shard: unit base + g + stride*m
    ubases = np.array([_ubase(u) for u in range(NU + 1)])
    ustrides = np.diff(ubases) // G
    stride = ustrides[tu]
    base = ubases[tu]
    cols = base[..., None] + tgg[..., None] + \
        stride[..., None] * np.arange(G)[None, None, :]          # [B, TOPG, G]
    valid = cols < SH
    grow_ = tcore[..., None] * SH + np.where(valid, cols, 0)     # [B, TOPG, G]

    cand_rows = grow_.reshape(B, TOPG * G)
    cand_valid = valid.reshape(B, TOPG * G)

    # exact rescore (fp32): sims = kn[rows] . qr  (chunked over queries)
    sims = np.full((B, TOPG * G), -np.inf, np.float32)
    CH = 64
    for lo in range(0, B, CH):
        hi = lo + CH
        kr = kn[cand_rows[lo:hi]]                                # [CH, T*G, D]
        sims[lo:hi] = np.einsum("qkd,qd->qk", kr, qr_full[lo:hi],
                                optimize=True)
    sims[~cand_valid] = -np.inf

    # tie rescue: a duplicated index among a unit's 8 slots near the cut
    # means fp16 value ties may have hidden a distinct group -> rescore unit.
    i8all = gidx                                    # [B, NCORES, NU*8]
    rescued = {}
    v48 = -np.sort(-fvals, axis=1)[:, TOPG - 1]
    for qi, ci, ui in zip(*_find_dup_units(i8all, vals, v48)):
        rows_lo = ci * SH + _ubase(ui)
        rows_hi = min(ci * SH + _ubase(ui + 1), ci * SH + SH)
        if rows_hi <= rows_lo:
            continue
        rws = np.arange(rows_lo, rows_hi)
        s = kn[rws] @ qr_full[qi]
        rescued.setdefault(qi, []).append((rws, s))

    top_idx = np.empty((B, K), np.int64)
    order = np.argpartition(-sims, K - 1, axis=1)[:, :K]
    for qi in range(B):
        if qi in rescued:
            rws = np.concatenate([cand_rows[qi]] + [r for r, _ in rescued[qi]])
            svs = np.concatenate([sims[qi]] + [s for _, s in rescued[qi]])
            uniq, first = np.unique(rws, return_index=True)
            svals = np.full(uniq.shape, -np.inf, np.float32)
            np.maximum.at(svals, np.searchsorted(uniq, rws), svs)
            sel = np.argpartition(-svals, K - 1)[:K]
            top_idx[qi] = uniq[sel]
        else:
            top_idx[qi] = np.take_along_axis(cand_rows[qi], order[qi], 0)

    # ---- phase 2 ----
    if "r2" not in _cache:
        _cache["r2"] = _SpmdRunner(_build_phase2())
    r2 = _cache["r2"]

    wm_a = Wm.reshape(4, 128, AU).transpose(1, 0, 2)   # [128, 4, 256] fp32

    Wc_ma = gam[:, None] * Wc[512:768]                 # [256, C] gamma-folded
    bc_eff = bc_ + bet @ Wc[512:768]                   # beta folded into bias
    srow = Wc_ma.sum(axis=0).astype(np.float32)        # [C]

    blob = np.zeros((128, BLOB_W), np.float32)
    blob[:, :1024] = Wq.reshape(4, 128, AU).transpose(1, 0, 2).reshape(128, 1024)
    wc_a = np.zeros((7, 128, C), np.float32)
    wc_a[:4] = Wc[:512].reshape(4, 128, C)
    wc_a[4, 0] = bc_eff
    wc_a[5:7] = Wc_ma.reshape(2, 128, C)
    blob[:, BLOB_WC:BLOB_WC + 700] = wc_a.transpose(1, 0, 2).reshape(128, 700)
    blob[:, BLOB_ID:BLOB_ID + 128] = np.eye(128, dtype=np.float32)
    cst = np.empty((128, 4), np.float32)
    cst[:, 0:2] = bq.reshape(2, 128).T
    cst[:, 2:4] = bm.reshape(2, 128).T

    wmk_cc = np.empty((NCORES, 128, 4, AU + NK), BF16NP)
    wmk_cc[:, :, :, :AU] = wm_a.astype(BF16NP)[None]
    qta_cc = np.empty((NCORES, 128, DC5, BQ), np.float32)
    for c in range(NCORES):
        flat = top_idx[c * BQ:(c + 1) * BQ].reshape(NK)
        wmk_cc[c, :, :, AU:] = keys[flat].T.reshape(4, 128, NK).transpose(
            1, 0, 2).astype(BF16NP)
        q_aug = np.zeros((DC5 * 128, BQ), np.float32)
        q_aug[:D] = qr_full[c * BQ:(c + 1) * BQ].T
        q_aug[D] = 1.0
        qta_cc[c] = q_aug.reshape(DC5, 128, BQ).transpose(1, 0, 2)

    res2 = r2({
        "wmk": wmk_cc.reshape(NCORES * 128, 4, AU + NK),
        "qta": qta_cc.reshape(NCORES * 128, DC5, BQ),
        "blob": _rep(blob), "srow": _rep(srow), "cst": _rep(cst),
    })

    out = np.concatenate([res2[c]["out"] for c in range(NCORES)], axis=0)
    return out.astype(np.float32)


def _find_dup_units(gidx, vals, v48):
    """(q, core, unit) triples whose 8 slots contain a duplicated index with
    value above the rescore cut (v48 - margin)."""
    Bq, NC, _ = gidx.shape
    g = gidx.reshape(Bq, NC, NU, 8)
    v = vals.reshape(Bq, NC, NU, 8)
    gs = np.sort(g, axis=3)
    dup = (np.diff(gs, axis=3) == 0).any(axis=3)          # [B, NC, NU]
    vmax = v.max(axis=3)
    margin = 8.0
    hit = dup & (vmax >= (v48[:, None, None] - margin))
    return np.nonzero(hit)



# revision 50
# speedup vs baseline: 1.1326x; 1.1326x over previous
"""Trainium2 Bass kernel for nn_MA_73478300500338 (retrieval_knn).

Pipeline (reference semantics):
  q = relu(query_embedding)                      [B, D]
  sim = cos(q, memory_keys); idx = top_k(sim, 32)
  mk = memory_keys[idx]
  qt = relu(q @ Wq + bq); mt = relu(mk @ Wm + bm)
  attended = sum_j mt[:, j, :]   (softmax over size-1 axis == 1)
  ma = LN(attended + qt) * gamma + beta
  out = [q, ma] @ Wc + bc                        [B, C]

Distribution (8 NeuronCores):
  Phase 1 (candidate scan): memory bank sharded 8x (12500 rows/core, padded
    to 13312). Keys are L2-normalized on host (ranking-invariant), scaled and
    cast to fp8e4m3. Each core computes all 256 queries x 13312 dots with
    fp8 DoubleRow matmuls (2 contraction rows/partition, 0.5 cyc/row), then
    selects candidates: Act copies PSUM->SBUF fp16, DVE does 3 contiguous
    pairwise-max rounds (group size 8) + Max8/MaxIndex per unit (units of
    1024/2048 keys). That yields top-8 groups-of-8 per unit.
  Host: merges 64 x 8 candidate groups/query, exactly rescores the members
    of the TOPG best groups in fp32 (and any unit that reported a
    duplicated index near the cut, to be robust to fp16 value ties and fp8
    dot noise), picks the exact top-32. Host work is pure indexing plus a
    small (256 x ~1024 x 512) batched dot product.
  Phase 2 (attention MLP): queries sharded 8x (32/core). mtT = relu(Wm^T mk
    + bm) runs in bf16 with au on partitions (per-partition bias), the sum
    over the 32 retrieved keys runs on DVE tensor_reduce, qt and the output
    projection run in fp32 (no separate Ldweights), and the entire
    layernorm affine (gamma, beta, mean, rstd) is folded into the output
    projection: out = q @ Wc_q + bc' + rstd*(x @ (gamma*Wc_ma) - mu*S).
"""

import os
import sys
import json

import numpy as np
import ml_dtypes

os.environ.setdefault("MYCRO_LOCAL_CACHE", "1")
if "/opt/trn_rl_repo" not in sys.path:
    sys.path.insert(0, "/opt/trn_rl_repo")

try:
    import jax as _jax
    _jax.config.update("jax_compilation_cache_dir", "/tmp/jax_cache_nn_ma")
    _jax.config.update("jax_persistent_cache_min_entry_size_bytes", -1)
    _jax.config.update("jax_persistent_cache_min_compile_time_secs", 0.5)
except Exception:
    pass

import bass_rust
import concourse.bass as bass
import concourse.bacc as bacc
import concourse.mybir as mybir
import concourse.tile as tile
from concourse.vector_clock import ScopedClock

# ---------------------------------------------------------------------------
# Workaround: this walrus build supports a single sync-wait per CTRL
# instruction, but Tile's stock tail drain carries one wait per busy
# processor. Split them into standalone single-wait instructions.
# ---------------------------------------------------------------------------


def _patched_drain_and_barrier(self, tick_clock, wait_clock):
    nc = self.nc
    with nc.discard():
        probe = nc.sync.drain()
        wait_clock.add_sem_waits(
            probe.ins, ScopedClock({None: tick_clock.global_clock})
        )
        j = json.loads(nc.instruction_to_json(probe.ins))
    waits = (j.get("sync_info") or {}).get("on_wait") or []
    for w in waits:
        sem = bass_rust.SemaphoreHandle(w["ant_name"], w["id"])
        assert w["wait_mode"] == "sem-ge-imm", w
        nc.sync.wait_ge(sem, w["wait_value"])
    nc.sync.drain()
    nc.all_engine_barrier()
    popped = nc._tile_sem_poison_stack.pop()
    assert popped is self._sem_poison
    nc.clear_and_free_semaphores(list(self.sems.allocated().values()))
    nc.all_engine_barrier()


tile.TileContext._drain_and_barrier = _patched_drain_and_barrier

# ---------------------------------------------------------------------------
# Problem shapes (hardcoded per spec)
# ---------------------------------------------------------------------------
B, N, D = 256, 100000, 512
AU, C, K = 256, 100, 32
NCORES = 8
SH = N // NCORES            # 12500 keys per core
SHP = 12800                 # padded shard width (25 x 512)
G = 8                       # group size (keys per candidate group)
TOPG = 128                  # groups rescored exactly per query
KSCALE = 64.0               # fp8 key scale (ranking-invariant)
EPS_LN = 1e-5

# Selection blocks: each block is one PSUM tile with exactly ONE drain
# consumer, so the matmul double-buffer never waits on a slow cascade.
#   "A": Act copies PSUM->fp16 SBUF (only Act can drain PSUM cheaply and
#        vector ops may read at most one PSUM operand); DVE runs 3
#        pairwise-halving max rounds into the group-maxima tile. The
#        GpSimd/Pool engine has no legal tensor-math opcodes on this
#        toolchain, so Act+DVE carry everything.
#   "D": one DVE tensor_reduce (window-8 max) drains PSUM straight into
#        the group-maxima tile; groups are 8 CONTIGUOUS columns here
# (width, path, spare)
BLOCKS = [(1024, "A", ""), (1024, "D", ""), (1024, "A", ""),
          (1024, "A", ""), (1024, "D", ""), (1024, "A", ""),
          (1024, "A", ""), (1024, "D", ""), (1024, "A", ""),
          (1024, "A", ""), (1024, "A", ""), (1024, "A", ""),
          (512, "A", "")]
NGTOT = sum(b[0] for b in BLOCKS) // G     # 1600 group maxima per bc
assert sum(b[0] for b in BLOCKS) == SHP
# DMA chunk widths for the key shard, 1:1 with blocks
KCHUNKS = [b[0] for b in BLOCKS]


def _subunits():
    """(g8s_offset, ngroups, col_base, width, path) per block.

    A-blocks: r1 pairwise-halving then window-4 pool: member m of group g
    is at col_base + 4g + (m & 3) + (m >> 2)*(w/2).
    D-blocks: contiguous window-8 groups: col_base + 8g + m.
    """
    out = []
    off = 0
    col = 0
    for w, p, _e in BLOCKS:
        ng = w // G
        out.append((off, ng, col, w, p))
        off += ng
        col += w
    assert off == NGTOT and col == SHP
    return out


def _slot_members():
    """[NGTOT, G] shard-column index of each group member, in g8s order."""
    cols = np.zeros((NGTOT, G), np.int64)
    m = np.arange(G)
    for off, ng, base, w, p in _subunits():
        g = np.arange(ng)[:, None]
        if p == "A":
            # 3 pairwise-halving rounds: member m of group g at stride ng
            cols[off:off + ng] = base + g + ng * m[None, :]
        else:
            cols[off:off + ng] = base + G * g + m[None, :]
    return cols

F32 = mybir.dt.float32
F16 = mybir.dt.float16
BF16 = mybir.dt.bfloat16
FP8 = mybir.dt.float8e4
U16 = mybir.dt.uint16
F8NP = ml_dtypes.float8_e4m3
BF16NP = ml_dtypes.bfloat16


_cache = {}


# ---------------------------------------------------------------------------
# Phase 1: fp8 DoubleRow dots + group-of-16 maxima shipped to host
# ---------------------------------------------------------------------------


def _build_phase1():
    nc = bacc.Bacc()
    q8d = nc.dram_tensor("q8", [128, 2, 2, B], FP8, kind="ExternalInput")
    k8d = nc.dram_tensor("k8", [128, 2, 2, SHP], FP8, kind="ExternalInput")
    t8 = nc.dram_tensor("t8", [2, 128, NGTOT], F16, kind="ExternalOutput")

    subs = _subunits()          # g8s layout, shared with the host decode
    sub_idx = 0

    with tile.TileContext(nc) as tc:
        with (
            tc.tile_pool(name="persist", bufs=1) as persist,
            tc.tile_pool(name="work", bufs=3) as wp,
            tc.tile_pool(name="psum", bufs=2, space="PSUM") as psump,
        ):
            # Copy-table preload (overlaps the initial DMA wait)
            dum = wp.tile([1, 2], F32, tag="dum")
            nc.vector.memset(dum, 0.0)
            nc.scalar.copy(out=dum[:, 1:2], in_=dum[:, 0:1])

            q8 = persist.tile([128, 2, 2, B], FP8)
            nc.sync.dma_start(out=q8, in_=q8d[:, :, :, :])
            k8 = persist.tile([128, 2, 2, SHP], FP8)
            col = 0
            for cw in KCHUNKS:
                nc.sync.dma_start(out=k8[:, :, :, col:col + cw],
                                  in_=k8d[:, :, :, col:col + cw])
                col += cw

            g8s = persist.tile([128, 2, NGTOT], F16)

            # PE p-state warm-up: spin the array during the DMA preamble
            wrm = persist.tile([128, 64], FP8)
            nc.gpsimd.memset(wrm, 0.0)
            for _ in range(10):
                psw = psump.tile([128, 1024], F32, tag="psA")
                nc.tensor.matmul(psw[:1, :64], wrm[:, :1], wrm[:, :],
                                 start=True, stop=True)

            def cascade(bi, bc, lo, w, path, ps):
                """Build the selection for one (block, bc) instance in two
                emission stages (early / mid). The driver emits early(i)
                right after the matmuls and mid(i) one instance later, so
                cross-engine inputs are old and in-order queues don't
                stall."""
                off, ng, base, _w, _p = subs[bi]
                assert base == lo and ng == w // G
                mid = []
                h = w // 2
                if path == "A":
                    dw = wp.tile([128, 1024], F16, tag=f"dw{bc}")
                    a1 = wp.tile([128, 512], F16, tag=f"a1{bc}")
                    a2 = wp.tile([128, 256], F16, tag=f"a2{bc}")
                    def early(dw=dw, ps=ps, w=w):
                        nc.scalar.copy(out=dw[:, :w], in_=ps[:, :w])
                    mid.append(lambda a1=a1, dw=dw, h=h, w=w:
                               nc.vector.tensor_max(out=a1[:, :h],
                                                    in0=dw[:, :h],
                                                    in1=dw[:, h:w]))
                    mid.append(lambda a2=a2, a1=a1, h=h:
                               nc.vector.tensor_max(
                                   out=a2[:, :h // 2], in0=a1[:, :h // 2],
                                   in1=a1[:, h // 2:h]))
                    mid.append(lambda a2=a2, h=h, off=off, ng=ng, bc=bc:
                               nc.vector.tensor_max(
                                   out=g8s[:, bc, off:off + ng],
                                   in0=a2[:, :h // 4],
                                   in1=a2[:, h // 4:h // 2]))
                else:
                    def early(ps=ps, w=w, off=off, ng=ng, bc=bc):
                        nc.vector.tensor_reduce(
                            out=g8s[:, bc, off:off + ng],
                            in_=ps[:, :w].rearrange("p (g j) -> p g j", j=G),
                            axis=mybir.AxisListType.X,
                            op=mybir.AluOpType.max)
                return early, mid

            # out-DMA split: ship completed g8s column ranges while the last
            # blocks still compute, leaving only a small final transfer
            osplit = subs[11][0]        # g8s col where the last 2 blocks start

            pend_mid = []
            lo = 0
            for bi, (w, path, _sp) in enumerate(BLOCKS):
                for bc in range(2):
                    # separate PSUM rings per drain path: a lagging Act copy
                    # must never stall a D-block's matmuls (and vice versa)
                    ps = psump.tile([128, 1024], F32, tag=f"ps{path}")
                    for w2 in range(w // 512):
                        for kc in range(2):
                            nc.tensor.matmul(
                                ps[:, w2 * 512:(w2 + 1) * 512],
                                q8[:, kc, :, bc * 128:(bc + 1) * 128],
                                k8[:, kc, :, lo + w2 * 512:lo + (w2 + 1) * 512],
                                start=(kc == 0), stop=(kc == 1),
                                perf_mode=mybir.MatmulPerfMode.DoubleRow,
                            )
                    early, mid = cascade(bi, bc, lo, w, path, ps)
                    early()
                    for f in pend_mid:
                        f()
                    pend_mid = mid
                    if bi == 12 and bc == 0:
                        # blocks 0..10 fully done by now (mid lag of 1)
                        nc.sync.dma_start(
                            out=bass.AP(t8, 0, [[NGTOT, 128],
                                                [128 * NGTOT, 2],
                                                [1, osplit]]),
                            in_=g8s[:, :, :osplit])
                lo += w
            for f in pend_mid:
                f()

            nc.sync.dma_start(
                out=bass.AP(t8, osplit, [[NGTOT, 128], [128 * NGTOT, 2],
                                         [1, NGTOT - osplit]]),
                in_=g8s[:, :, osplit:])
    nc.finalize()
    return nc


# ---------------------------------------------------------------------------
# Phase 2: attention MLP + LN + output projection (32 queries per core, bf16)
# ---------------------------------------------------------------------------
BQ = B // NCORES            # 32 queries per core
NK = BQ * K                 # 1024 gathered key columns per core
DC5 = 5                     # 4 d-chunks + 1 bias-aug chunk


BLOB_WQ = 0                 # fp32 blob layout (columns)
BLOB_WC = 1024              # 7 chunks x 100
BLOB_ID = 1724              # identity 128
BLOB_W = 1852


def _build_phase2():
    nc = bacc.Bacc()
    cst_ = nc.dram_tensor("cst", [128, 4], F32, kind="ExternalInput")
    wmk_ = nc.dram_tensor("wmk", [128, 4, AU + NK], BF16, kind="ExternalInput")
    qta_ = nc.dram_tensor("qta", [128, DC5, BQ], F32, kind="ExternalInput")
    blob_ = nc.dram_tensor("blob", [128, BLOB_W], F32, kind="ExternalInput")
    srow_ = nc.dram_tensor("srow", [C], F32, kind="ExternalInput")
    out = nc.dram_tensor("out", [BQ, C], F32, kind="ExternalOutput")

    RELU = mybir.ActivationFunctionType.Relu
    with tile.TileContext(nc) as tc:
        with (
            tc.tile_pool(name="p", bufs=1) as pool,
            tc.tile_pool(name="psmt", bufs=3, space="PSUM") as psmt,
            tc.tile_pool(name="psq", bufs=1, space="PSUM") as psq,
            tc.tile_pool(name="ps1", bufs=1, space="PSUM") as ps1,
        ):
            # activation-table preloads (overlap the DMA wait)
            dum = pool.tile([1, 2], F32)
            nc.vector.memset(dum, 0.0)
            nc.scalar.activation(out=dum[:, 1:2], in_=dum[:, 0:1], func=RELU)
            nc.scalar.activation(out=dum[:, 0:1], in_=dum[:, 1:2],
                                 func=mybir.ActivationFunctionType.Sqrt)

            # PE p-state warm-up: keep the array busy through the DMA
            # preamble so the real matmuls run at peak clock
            wrm = pool.tile([128, 256], BF16)
            nc.vector.memset(wrm, 0.0)
            for w_ in range(24):
                pw = psq.tile([128, BQ], F32, tag="pqt")
                nc.tensor.matmul(pw[:1, :], wrm[:, :1], wrm[:, 128:128 + BQ],
                                 start=True, stop=True)

            # loads: Wm and mk arrive per contraction chunk so the matmul
            # stream starts after the first quarter
            wmk = pool.tile([128, 4, AU + NK], BF16)
            for c in range(4):
                nc.sync.dma_start(out=wmk[:, c, :], in_=wmk_[:, c, :])
            wm = wmk[:, :, :AU]
            mkt = wmk[:, :, AU:]
            cst = pool.tile([128, 4], F32)
            nc.sync.dma_start(out=cst, in_=cst_[:, :])
            qta = pool.tile([128, DC5, BQ], F32)
            nc.sync.dma_start(out=qta, in_=qta_[:, :, :])
            blob = pool.tile([128, BLOB_W], F32)
            nc.sync.dma_start(out=blob, in_=blob_[:, :])
            srt = pool.tile([BQ, C], F32)
            nc.sync.dma_start(out=srt, in_=bass.AP(srow_, 0, [[0, BQ], [1, C]]))

            # mtT[au, nk] = relu(Wm^T mk + bm); bias is per-partition here.
            # Matmuls are emitted chunk-major so the in-order PE queue tracks
            # the chunk DMA arrivals instead of serializing on the last one.
            mtT = pool.tile([128, 2, NK], BF16)
            attT = pool.tile([128, 2, BQ], BF16)
            groups = [(a, h) for a in range(2) for h in range(2)]
            pmts = []
            for _gi in range(len(groups)):
                pmt = psmt.tile([128, NK // 2], F32, tag="pmt")
                pmts.append(pmt)
            for c in range(4):
                for gi, (a, h) in enumerate(groups):
                    nc.tensor.matmul(
                        pmts[gi], wm[:, c, a * 128:(a + 1) * 128],
                        mkt[:, c, h * 512:(h + 1) * 512],
                        start=(c == 0), stop=(c == 3))
            for gi, (a, h) in enumerate(groups):
                nc.scalar.activation(
                    out=mtT[:, a, h * 512:(h + 1) * 512], in_=pmts[gi],
                    func=RELU, bias=cst[:, 2 + a:3 + a], scale=1.0)
                # attT[au, b] = sum_j mtT[au, (b j)] on DVE (fp32 internal)
                with nc.allow_low_precision(
                        reason="DVE reduces in fp32 internally; bf16 "
                               "output rounding is ~0.4% on 2e-2 tol"):
                    nc.vector.tensor_reduce(
                        out=attT[:, a, h * 16:(h + 1) * 16],
                        in_=mtT[:, a, h * 512:(h + 1) * 512].rearrange(
                            "p (b j) -> p b j", j=K),
                        axis=mybir.AxisListType.X, op=mybir.AluOpType.add)

            # qtT[au, b] = relu(Wq^T q + bq), fp32 matmuls (no ldweights)
            xT = pool.tile([128, 2, BQ], F32)
            for a in range(2):
                pqt = psq.tile([128, BQ], F32, tag="pqt")
                for c in range(4):
                    nc.tensor.matmul(
                        pqt, blob[:, BLOB_WQ + c * AU + a * 128:
                                  BLOB_WQ + c * AU + (a + 1) * 128],
                        qta[:, c, :], start=(c == 0), stop=(c == 3))
                qts = pool.tile([128, BQ], F32, tag=f"qts{a}")
                nc.scalar.activation(
                    out=qts, in_=pqt, func=RELU,
                    bias=cst[:, a:a + 1], scale=1.0)
                nc.vector.tensor_add(out=xT[:, a, :], in0=attT[:, a, :],
                                     in1=qts)

            # transpose xT -> x [b, au] (fp32)
            idt = blob[:, BLOB_ID:BLOB_ID + 128]
            x = pool.tile([BQ, AU], F32)
            st6 = pool.tile([BQ, 2, 6], F32)
            for a in range(2):
                pst = ps1.tile([BQ, 128], F32, tag="pst")
                nc.tensor.transpose(pst, xT[:, a, :], idt[:128, :128])
                nc.scalar.copy(out=x[:, a * 128:(a + 1) * 128], in_=pst)
                # layernorm stats per half, overlapping the other transpose;
                # gamma/beta and the (x-mu)*rstd affine are folded into Wc
                nc.vector.bn_stats(out=st6[:, a, :],
                                   in_=x[:, a * 128:(a + 1) * 128])
            st2 = pool.tile([BQ, 2], F32)
            nc.vector.bn_aggr(out=st2, in_=st6)
            eps = pool.tile([BQ, 1], F32)
            nc.vector.memset(eps, EPS_LN)
            sd = pool.tile([BQ, 1], F32)
            nc.scalar.activation(out=sd, in_=st2[:, 1:2],
                                 func=mybir.ActivationFunctionType.Sqrt,
                                 bias=eps, scale=1.0)
            rstd = pool.tile([BQ, 1], F32)
            nc.vector.reciprocal(out=rstd, in_=sd)

            # out = q @ Wc_q + bc' + rstd*(x @ Wc_ma' - mu*S); the x @ Wc_ma
            # product reads xT directly (x's transpose IS xT — the baseline's
            # round-trip transpose back to [au, b] was redundant)
            poq = ps1.tile([BQ, C], F32, tag="poq")
            for c in range(DC5):
                nc.tensor.matmul(
                    poq, qta[:, c, :],
                    blob[:, BLOB_WC + c * C:BLOB_WC + (c + 1) * C],
                    start=(c == 0), stop=(c == DC5 - 1))
            pom = ps1.tile([BQ, C], F32, tag="pom")
            for a in range(2):
                nc.tensor.matmul(
                    pom, xT[:, a, :],
                    blob[:, BLOB_WC + (5 + a) * C:BLOB_WC + (6 + a) * C],
                    start=(a == 0), stop=(a == 1))
            # out = poq + rstd*(pom - mu*S): f = mu*S runs early (only needs
            # mu), the rstd dependency lands in the final fused op
            f1 = pool.tile([BQ, C], F32)
            nc.vector.tensor_scalar(out=f1, in0=srt, scalar1=st2[:, 0:1],
                                    scalar2=None, op0=mybir.AluOpType.mult)
            e2 = pool.tile([BQ, C], F32)
            nc.vector.tensor_sub(out=e2, in0=pom, in1=f1)
            ot = pool.tile([BQ, C], F32)
            nc.vector.scalar_tensor_tensor(out=ot, in0=e2, scalar=rstd,
                                           in1=poq, op0=mybir.AluOpType.mult,
                                           op1=mybir.AluOpType.add)
            nc.sync.dma_start(out=out[:, :], in_=ot)
    nc.finalize()
    return nc


# ---------------------------------------------------------------------------
# SPMD runner with a persistent jitted executable
# ---------------------------------------------------------------------------


class _SpmdRunner:
    def __init__(self, nc, n_cores=NCORES):
        import jax
        from jax.sharding import Mesh, PartitionSpec
        from concourse.bass2jax import (
            _bass_exec_p,
            install_neuronx_cc_hook,
            partition_id_tensor,
        )

        try:
            from jax.experimental.shard_map import shard_map
        except ImportError:
            from jax.shard_map import shard_map

        install_neuronx_cc_hook()
        self.jax = jax
        partition_name = (
            nc.partition_id_tensor.name if nc.partition_id_tensor else None
        )
        in_names, out_names, out_avals, zero_outs = [], [], [], []
        for alloc in nc.m.functions[0].allocations:
            if not isinstance(alloc, mybir.MemoryLocationSet):
                continue
            name = alloc.memorylocations[0].name
            if alloc.kind == "ExternalInput":
                if name != partition_name:
                    in_names.append(name)
            elif alloc.kind == "ExternalOutput":
                shape = tuple(alloc.tensor_shape)
                dtype = mybir.dt.np(alloc.dtype)
                out_names.append(name)
                out_avals.append(jax.core.ShapedArray(shape, dtype))
                zero_outs.append(np.zeros((n_cores * shape[0], *shape[1:]), dtype))
        self.in_names = list(in_names)
        self.out_names = out_names
        self.out_avals = out_avals
        self.zero_outs = zero_outs
        self.n_cores = n_cores
        n_params = len(in_names)
        n_outs = len(out_names)
        all_in = in_names + out_names + ([partition_name] if partition_name else [])

        def _body(*args):
            operands = list(args)
            if partition_name is not None:
                operands.append(partition_id_tensor())
            return tuple(
                _bass_exec_p.bind(
                    *operands,
                    out_avals=tuple(out_avals),
                    in_names=tuple(all_in),
                    out_names=tuple(out_names),
                    lowering_input_output_aliases=(),
                    sim_require_finite=True,
                    sim_require_nnan=True,
                    nc=nc,
                )
            )

        devices = jax.devices()[:n_cores]
        mesh = Mesh(np.asarray(devices), ("core",))
        in_specs = (PartitionSpec("core"),) * (n_params + n_outs)
        out_specs = (PartitionSpec("core"),) * n_outs
        self.sharded = jax.jit(
            shard_map(
                _body, mesh=mesh, in_specs=in_specs, out_specs=out_specs,
                check_rep=False,
            ),
            donate_argnums=tuple(range(n_params, n_params + n_outs)),
            keep_unused=True,
        )

    def __call__(self, concat_in):
        args = [concat_in[n] for n in self.in_names]
        zeros = [np.zeros_like(z) for z in self.zero_outs]
        out_arrs = self.sharded(*args, *zeros)
        res = []
        for c in range(self.n_cores):
            res.append({
                name: np.asarray(out_arrs[i]).reshape(
                    self.n_cores, *self.out_avals[i].shape
                )[c]
                for i, name in enumerate(self.out_names)
            })
        return res


def _rep(a):
    a = np.ascontiguousarray(a)
    return np.broadcast_to(a, (NCORES,) + a.shape).reshape(
        NCORES * a.shape[0], *a.shape[1:]
    )


# ---------------------------------------------------------------------------
# Host orchestration
# ---------------------------------------------------------------------------


def kernel(**inputs):
    qe = np.asarray(inputs["query_embedding"], dtype=np.float32)
    keys = np.asarray(inputs["memory_keys"], dtype=np.float32)
    Wq = np.asarray(inputs["Wq"], dtype=np.float32)
    bq = np.asarray(inputs["bq"], dtype=np.float32)
    Wm = np.asarray(inputs["Wm"], dtype=np.float32)
    bm = np.asarray(inputs["bm"], dtype=np.float32)
    gam = np.asarray(inputs["ln_gamma"], dtype=np.float32)
    bet = np.asarray(inputs["ln_beta"], dtype=np.float32)
    Wc = np.asarray(inputs["Wc"], dtype=np.float32)
    bc_ = np.asarray(inputs["bc"], dtype=np.float32)
    k = int(inputs["k"])
    assert k == K and qe.shape == (B, D) and keys.shape == (N, D)

    # ---- host prep: normalize keys, fp8 layouts ----
    mn = np.sqrt(np.einsum("nd,nd->n", keys, keys, dtype=np.float64)).astype(np.float32)
    kn = keys * (1.0 / mn)[:, None]                 # [N, D] fp32, for rescoring
    qr_full = np.maximum(qe, 0.0)                   # [B, D] fp32 relu'd queries

    k8 = (kn * KSCALE).astype(F8NP)                 # [N, D] fp8
    q8T = np.ascontiguousarray(qr_full.T).astype(F8NP)   # [D, B] fp8
    q8_dev = q8T.reshape(2, 2, 128, B).transpose(2, 0, 1, 3)  # [128,2,2,B]

    import jax
    from jax.sharding import Mesh, NamedSharding, PartitionSpec
    devices = jax.devices()[:NCORES]
    mesh = Mesh(np.asarray(devices), ("core",))
    csh = NamedSharding(mesh, PartitionSpec("core"))

    parts = []
    for c in range(NCORES):
        kT = np.zeros((D, SHP), F8NP)
        kT[:, :SH] = k8[c * SH:(c + 1) * SH].T
        shard = np.ascontiguousarray(
            kT.reshape(2, 2, 128, SHP).transpose(2, 0, 1, 3))
        parts.append(jax.device_put(shard, devices[c]))
    k8_dev = jax.make_array_from_single_device_arrays(
        (NCORES * 128, 2, 2, SHP), csh, parts)

    if "r1" not in _cache:
        _cache["r1"] = _SpmdRunner(_build_phase1())
    res1 = _cache["r1"]({"q8": _rep(np.ascontiguousarray(q8_dev)), "k8": k8_dev})

    # ---- host merge: top-TOPG groups by fp16 group-max, exact rescore ----
    vals = np.empty((B, NCORES, NGTOT), np.float32)
    for c in range(NCORES):
        vals[:, c, :] = res1[c]["t8"].reshape(2 * 128, NGTOT)

    members = _slot_members()                   # [NGTOT, G] shard columns

    fvals = vals.reshape(B, NCORES * NGTOT)
    top = np.argpartition(-fvals, TOPG - 1, axis=1)[:, :TOPG]   # [B, TOPG]
    tcore, tslot = top // NGTOT, top % NGTOT
    cols = members[tslot]                                       # [B, TOPG, G]
    valid = cols < SH
    grow_ = tcore[..., None] * SH + np.where(valid, cols, 0)     # [B, TOPG, G]

    cand_rows = grow_.reshape(B, TOPG * G)
    cand_valid = valid.reshape(B, TOPG * G)

    # exact rescore (fp32): sims = kn[rows] . qr  (chunked over queries)
    sims = np.full((B, TOPG * G), -np.inf, np.float32)
    CH = 32
    for lo in range(0, B, CH):
        hi = lo + CH
        kr = kn[cand_rows[lo:hi]]                                # [CH, T*G, D]
        sims[lo:hi] = np.einsum("qkd,qd->qk", kr, qr_full[lo:hi],
                                optimize=True)
    sims[~cand_valid] = -np.inf

    order = np.argpartition(-sims, K - 1, axis=1)[:, :K]
    top_idx = np.take_along_axis(cand_rows, order, axis=1)       # [B, K]

    # ---- phase 2 ----
    if "r2" not in _cache:
        _cache["r2"] = _SpmdRunner(_build_phase2())
    r2 = _cache["r2"]

    wm_a = Wm.reshape(4, 128, AU).transpose(1, 0, 2)   # [128, 4, 256] fp32

    Wc_ma = gam[:, None] * Wc[512:768]                 # [256, C] gamma-folded
    bc_eff = bc_ + bet @ Wc[512:768]                   # beta folded into bias
    srow = Wc_ma.sum(axis=0).astype(np.float32)        # [C]

    blob = np.zeros((128, BLOB_W), np.float32)
    blob[:, :1024] = Wq.reshape(4, 128, AU).transpose(1, 0, 2).reshape(128, 1024)
    wc_a = np.zeros((7, 128, C), np.float32)
    wc_a[:4] = Wc[:512].reshape(4, 128, C)
    wc_a[4, 0] = bc_eff
    wc_a[5:7] = Wc_ma.reshape(2, 128, C)
    blob[:, BLOB_WC:BLOB_WC + 700] = wc_a.transpose(1, 0, 2).reshape(128, 700)
    blob[:, BLOB_ID:BLOB_ID + 128] = np.eye(128, dtype=np.float32)
    cst = np.empty((128, 4), np.float32)
    cst[:, 0:2] = bq.reshape(2, 128).T
    cst[:, 2:4] = bm.reshape(2, 128).T

    wmk_cc = np.empty((NCORES, 128, 4, AU + NK), BF16NP)
    wmk_cc[:, :, :, :AU] = wm_a.astype(BF16NP)[None]
    qta_cc = np.empty((NCORES, 128, DC5, BQ), np.float32)
    for c in range(NCORES):
        flat = top_idx[c * BQ:(c + 1) * BQ].reshape(NK)
        wmk_cc[c, :, :, AU:] = keys[flat].T.reshape(4, 128, NK).transpose(
            1, 0, 2).astype(BF16NP)
        q_aug = np.zeros((DC5 * 128, BQ), np.float32)
        q_aug[:D] = qr_full[c * BQ:(c + 1) * BQ].T
        q_aug[D] = 1.0
        qta_cc[c] = q_aug.reshape(DC5, 128, BQ).transpose(1, 0, 2)

    res2 = r2({
        "wmk": wmk_cc.reshape(NCORES * 128, 4, AU + NK),
        "qta": qta_cc.reshape(NCORES * 128, DC5, BQ),
        "blob": _rep(blob), "srow": _rep(srow), "cst": _rep(cst),
    })

    out = np.concatenate([res2[c]["out"] for c in range(NCORES)], axis=0)
    return out.astype(np.float32)



# revision 56
# speedup vs baseline: 1.1334x; 1.0007x over previous
"""Trainium2 Bass kernel for nn_MA_73478300500338 (retrieval_knn).

Pipeline (reference semantics):
  q = relu(query_embedding)                      [B, D]
  sim = cos(q, memory_keys); idx = top_k(sim, 32)
  mk = memory_keys[idx]
  qt = relu(q @ Wq + bq); mt = relu(mk @ Wm + bm)
  attended = sum_j mt[:, j, :]   (softmax over size-1 axis == 1)
  ma = LN(attended + qt) * gamma + beta
  out = [q, ma] @ Wc + bc                        [B, C]

Distribution (8 NeuronCores):
  Phase 1 (candidate scan): memory bank sharded 8x (12500 rows/core, padded
    to 12800). Keys are L2-normalized on host (ranking-invariant), scaled
    and cast to fp8e4m3. Each core computes all 256 queries x 12800 dots
    with fp8 DoubleRow matmuls (0.5 cyc/row), streaming the shard in
    per-block DMA chunks. Each block's PSUM tile has exactly ONE drain:
    "A" blocks: Act copies PSUM->fp16, DVE runs 3 pairwise-max rounds into
    the group-maxima tile (strided groups of 8); "D" blocks: one DVE
    window-8 tensor_reduce drains PSUM straight to group maxima
    (contiguous groups). All 1600 group-maxima/bc ship to the host.
  Host: picks the TOPG=128 best groups per query over all cores, exactly
    rescores their 1024 member rows in fp32, picks the exact top-32.
  Phase 2 (attention MLP): queries sharded 8x (32/core). mtT = relu(Wm^T mk
    + bm) runs in bf16 with au on partitions (relu alternating Act/DVE),
    the sum over the 32 retrieved keys runs on DVE tensor_reduce, qt and
    the output projection run in fp32, and the entire layernorm affine
    (gamma, beta, mean, rstd) is folded into the output projection:
    out = q @ Wc_q + bc' + rstd*(x @ (gamma*Wc_ma) - mu*S), with the
    x @ Wc_ma product reading the [au, b]-layout xT directly.
"""

import os
import sys
import json

import numpy as np
import ml_dtypes

os.environ.setdefault("MYCRO_LOCAL_CACHE", "1")
if "/opt/trn_rl_repo" not in sys.path:
    sys.path.insert(0, "/opt/trn_rl_repo")

try:
    import jax as _jax
    _jax.config.update("jax_compilation_cache_dir", "/tmp/jax_cache_nn_ma")
    _jax.config.update("jax_persistent_cache_min_entry_size_bytes", -1)
    _jax.config.update("jax_persistent_cache_min_compile_time_secs", 0.5)
except Exception:
    pass

import bass_rust
import concourse.bass as bass
import concourse.bacc as bacc
import concourse.mybir as mybir
import concourse.tile as tile
from concourse.vector_clock import ScopedClock

# ---------------------------------------------------------------------------
# Workaround: this walrus build supports a single sync-wait per CTRL
# instruction, but Tile's stock tail drain carries one wait per busy
# processor. Split them into standalone single-wait instructions.
# ---------------------------------------------------------------------------


def _patched_drain_and_barrier(self, tick_clock, wait_clock):
    nc = self.nc
    with nc.discard():
        probe = nc.sync.drain()
        wait_clock.add_sem_waits(
            probe.ins, ScopedClock({None: tick_clock.global_clock})
        )
        j = json.loads(nc.instruction_to_json(probe.ins))
    waits = (j.get("sync_info") or {}).get("on_wait") or []
    for w in waits:
        sem = bass_rust.SemaphoreHandle(w["ant_name"], w["id"])
        assert w["wait_mode"] == "sem-ge-imm", w
        nc.sync.wait_ge(sem, w["wait_value"])
    nc.sync.drain()
    nc.all_engine_barrier()
    popped = nc._tile_sem_poison_stack.pop()
    assert popped is self._sem_poison
    nc.clear_and_free_semaphores(list(self.sems.allocated().values()))
    nc.all_engine_barrier()


tile.TileContext._drain_and_barrier = _patched_drain_and_barrier

# ---------------------------------------------------------------------------
# Problem shapes (hardcoded per spec)
# ---------------------------------------------------------------------------
B, N, D = 256, 100000, 512
AU, C, K = 256, 100, 32
NCORES = 8
SH = N // NCORES            # 12500 keys per core
SHP = 12800                 # padded shard width (25 x 512)
G = 8                       # group size (keys per candidate group)
TOPG = 128                  # groups rescored exactly per query
KSCALE = 64.0               # fp8 key scale (ranking-invariant)
EPS_LN = 1e-5

# Selection blocks: each block is one PSUM tile with exactly ONE drain
# consumer, so the matmul double-buffer never waits on a slow cascade.
#   "A": Act copies PSUM->fp16 SBUF (only Act can drain PSUM cheaply and
#        vector ops may read at most one PSUM operand); DVE runs 3
#        pairwise-halving max rounds into the group-maxima tile. The
#        GpSimd/Pool engine has no legal tensor-math opcodes on this
#        toolchain, so Act+DVE carry everything.
#   "D": one DVE tensor_reduce (window-8 max) drains PSUM straight into
#        the group-maxima tile; groups are 8 CONTIGUOUS columns here
# (width, path, spare)
BLOCKS = [(1024, "A", ""), (1024, "D", ""), (1024, "A", ""),
          (1024, "A", ""), (1024, "D", ""), (1024, "A", ""),
          (1024, "A", ""), (1024, "D", ""), (1024, "A", ""),
          (1024, "A", ""), (1024, "A", ""), (1024, "A", ""),
          (512, "A", "")]
NGTOT = sum(b[0] for b in BLOCKS) // G     # 1600 group maxima per bc
assert sum(b[0] for b in BLOCKS) == SHP
# DMA chunk widths for the key shard, 1:1 with blocks
KCHUNKS = [b[0] for b in BLOCKS]


def _subunits():
    """(g8s_offset, ngroups, col_base, width, path) per block.

    A-blocks: r1 pairwise-halving then window-4 pool: member m of group g
    is at col_base + 4g + (m & 3) + (m >> 2)*(w/2).
    D-blocks: contiguous window-8 groups: col_base + 8g + m.
    """
    out = []
    off = 0
    col = 0
    for w, p, _e in BLOCKS:
        ng = w // G
        out.append((off, ng, col, w, p))
        off += ng
        col += w
    assert off == NGTOT and col == SHP
    return out


def _slot_members():
    """[NGTOT, G] shard-column index of each group member, in g8s order."""
    cols = np.zeros((NGTOT, G), np.int64)
    m = np.arange(G)
    for off, ng, base, w, p in _subunits():
        g = np.arange(ng)[:, None]
        if p == "A":
            # 3 pairwise-halving rounds: member m of group g at stride ng
            cols[off:off + ng] = base + g + ng * m[None, :]
        else:
            cols[off:off + ng] = base + G * g + m[None, :]
    return cols

F32 = mybir.dt.float32
F16 = mybir.dt.float16
BF16 = mybir.dt.bfloat16
FP8 = mybir.dt.float8e4
U16 = mybir.dt.uint16
F8NP = ml_dtypes.float8_e4m3
BF16NP = ml_dtypes.bfloat16


_cache = {}


# ---------------------------------------------------------------------------
# Phase 1: fp8 DoubleRow dots + group-of-16 maxima shipped to host
# ---------------------------------------------------------------------------


def _build_phase1():
    nc = bacc.Bacc()
    q8d = nc.dram_tensor("q8", [128, 2, 2, B], FP8, kind="ExternalInput")
    k8d = nc.dram_tensor("k8", [128, 2, 2, SHP], FP8, kind="ExternalInput")
    t8 = nc.dram_tensor("t8", [2, 128, NGTOT], F16, kind="ExternalOutput")

    subs = _subunits()          # g8s layout, shared with the host decode
    sub_idx = 0

    with tile.TileContext(nc) as tc:
        with (
            tc.tile_pool(name="persist", bufs=1) as persist,
            tc.tile_pool(name="work", bufs=3) as wp,
            tc.tile_pool(name="psum", bufs=2, space="PSUM") as psump,
        ):
            # Copy-table preload (overlaps the initial DMA wait)
            dum = wp.tile([1, 2], F32, tag="dum")
            nc.vector.memset(dum, 0.0)
            nc.scalar.copy(out=dum[:, 1:2], in_=dum[:, 0:1])

            q8 = persist.tile([128, 2, 2, B], FP8)
            nc.sync.dma_start(out=q8, in_=q8d[:, :, :, :])
            k8 = persist.tile([128, 2, 2, SHP], FP8)
            col = 0
            for cw in KCHUNKS:
                nc.sync.dma_start(out=k8[:, :, :, col:col + cw],
                                  in_=k8d[:, :, :, col:col + cw])
                col += cw

            g8s = persist.tile([128, 2, NGTOT], F16)

            # PE p-state warm-up: spin the array during the DMA preamble
            wrm = persist.tile([128, 64], FP8)
            nc.gpsimd.memset(wrm, 0.0)
            for _ in range(10):
                psw = psump.tile([128, 1024], F32, tag="psA")
                nc.tensor.matmul(psw[:1, :64], wrm[:, :1], wrm[:, :],
                                 start=True, stop=True)

            def cascade(bi, bc, lo, w, path, ps):
                """Build the selection for one (block, bc) instance in two
                emission stages (early / mid). The driver emits early(i)
                right after the matmuls and mid(i) one instance later, so
                cross-engine inputs are old and in-order queues don't
                stall."""
                off, ng, base, _w, _p = subs[bi]
                assert base == lo and ng == w // G
                mid = []
                h = w // 2
                if path == "A":
                    dw = wp.tile([128, 1024], F16, tag=f"dw{bc}")
                    a1 = wp.tile([128, 512], F16, tag=f"a1{bc}")
                    a2 = wp.tile([128, 256], F16, tag=f"a2{bc}")
                    def early(dw=dw, ps=ps, w=w):
                        nc.scalar.copy(out=dw[:, :w], in_=ps[:, :w])
                    mid.append(lambda a1=a1, dw=dw, h=h, w=w:
                               nc.vector.tensor_max(out=a1[:, :h],
                                                    in0=dw[:, :h],
                                                    in1=dw[:, h:w]))
                    mid.append(lambda a2=a2, a1=a1, h=h:
                               nc.vector.tensor_max(
                                   out=a2[:, :h // 2], in0=a1[:, :h // 2],
                                   in1=a1[:, h // 2:h]))
                    mid.append(lambda a2=a2, h=h, off=off, ng=ng, bc=bc:
                               nc.vector.tensor_max(
                                   out=g8s[:, bc, off:off + ng],
                                   in0=a2[:, :h // 4],
                                   in1=a2[:, h // 4:h // 2]))
                else:
                    def early(ps=ps, w=w, off=off, ng=ng, bc=bc):
                        nc.vector.tensor_reduce(
                            out=g8s[:, bc, off:off + ng],
                            in_=ps[:, :w].rearrange("p (g j) -> p g j", j=G),
                            axis=mybir.AxisListType.X,
                            op=mybir.AluOpType.max)
                return early, mid

            # out-DMA split: ship completed g8s column ranges while the last
            # blocks still compute, leaving only a small final transfer
            osplit = subs[11][0]        # g8s col where the last 2 blocks start

            pend_mid = []
            lo = 0
            for bi, (w, path, _sp) in enumerate(BLOCKS):
                for bc in range(2):
                    # separate PSUM rings per drain path: a lagging Act copy
                    # must never stall a D-block's matmuls (and vice versa)
                    ps = psump.tile([128, 1024], F32, tag=f"ps{path}")
                    for w2 in range(w // 512):
                        for kc in range(2):
                            nc.tensor.matmul(
                                ps[:, w2 * 512:(w2 + 1) * 512],
                                q8[:, kc, :, bc * 128:(bc + 1) * 128],
                                k8[:, kc, :, lo + w2 * 512:lo + (w2 + 1) * 512],
                                start=(kc == 0), stop=(kc == 1),
                                perf_mode=mybir.MatmulPerfMode.DoubleRow,
                            )
                    early, mid = cascade(bi, bc, lo, w, path, ps)
                    early()
                    for f in pend_mid:
                        f()
                    pend_mid = mid
                    if bi == 12 and bc == 0:
                        # blocks 0..10 fully done by now (mid lag of 1)
                        nc.sync.dma_start(
                            out=bass.AP(t8, 0, [[NGTOT, 128],
                                                [128 * NGTOT, 2],
                                                [1, osplit]]),
                            in_=g8s[:, :, :osplit])
                lo += w
            for f in pend_mid:
                f()

            nc.sync.dma_start(
                out=bass.AP(t8, osplit, [[NGTOT, 128], [128 * NGTOT, 2],
                                         [1, NGTOT - osplit]]),
                in_=g8s[:, :, osplit:])
    nc.finalize()
    return nc


# ---------------------------------------------------------------------------
# Phase 2: attention MLP + LN + output projection (32 queries per core, bf16)
# ---------------------------------------------------------------------------
BQ = B // NCORES            # 32 queries per core
NK = BQ * K                 # 1024 gathered key columns per core
DC5 = 5                     # 4 d-chunks + 1 bias-aug chunk


BLOB_WQ = 0                 # fp32 blob layout (columns)
BLOB_WC = 1024              # 7 chunks x 100
BLOB_ID = 1724              # identity 128
BLOB_W = 1852


def _build_phase2():
    nc = bacc.Bacc()
    cst_ = nc.dram_tensor("cst", [128, 4], F32, kind="ExternalInput")
    wmk_ = nc.dram_tensor("wmk", [128, 4, AU + NK], BF16, kind="ExternalInput")
    qta_ = nc.dram_tensor("qta", [128, DC5, BQ], F32, kind="ExternalInput")
    blob_ = nc.dram_tensor("blob", [128, BLOB_W], F32, kind="ExternalInput")
    srow_ = nc.dram_tensor("srow", [C], F32, kind="ExternalInput")
    out = nc.dram_tensor("out", [BQ, C], F32, kind="ExternalOutput")

    RELU = mybir.ActivationFunctionType.Relu
    with tile.TileContext(nc) as tc:
        with (
            tc.tile_pool(name="p", bufs=1) as pool,
            tc.tile_pool(name="psmt", bufs=3, space="PSUM") as psmt,
            tc.tile_pool(name="psq", bufs=1, space="PSUM") as psq,
            tc.tile_pool(name="ps1", bufs=1, space="PSUM") as ps1,
        ):
            # activation-table preloads (overlap the DMA wait)
            dum = pool.tile([1, 2], F32)
            nc.vector.memset(dum, 0.0)
            nc.scalar.activation(out=dum[:, 1:2], in_=dum[:, 0:1], func=RELU)
            nc.scalar.activation(out=dum[:, 0:1], in_=dum[:, 1:2],
                                 func=mybir.ActivationFunctionType.Sqrt)

            # PE p-state warm-up: keep the array busy through the DMA
            # preamble so the real matmuls run at peak clock
            wrm = pool.tile([128, 256], BF16)
            nc.vector.memset(wrm, 0.0)
            for w_ in range(10):
                pw = psq.tile([128, BQ], F32, tag="pqt")
                nc.tensor.matmul(pw[:1, :], wrm[:, :1], wrm[:, 128:128 + BQ],
                                 start=True, stop=True)

            # loads: Wm and mk arrive per contraction chunk so the matmul
            # stream starts after the first quarter
            wmk = pool.tile([128, 4, AU + NK], BF16)
            for c in range(4):
                nc.sync.dma_start(out=wmk[:, c, :], in_=wmk_[:, c, :])
            wm = wmk[:, :, :AU]
            mkt = wmk[:, :, AU:]
            cst = pool.tile([128, 4], F32)
            nc.sync.dma_start(out=cst, in_=cst_[:, :])
            qta = pool.tile([128, DC5, BQ], F32)
            nc.sync.dma_start(out=qta, in_=qta_[:, :, :])
            blob = pool.tile([128, BLOB_W], F32)
            # Wq columns land first so the qt path unblocks ~1.2us earlier;
            # Wc/identity follow in a second transfer
            nc.sync.dma_start(out=blob[:, :BLOB_WC], in_=blob_[:, :BLOB_WC])
            nc.sync.dma_start(out=blob[:, BLOB_WC:], in_=blob_[:, BLOB_WC:])
            srt = pool.tile([BQ, C], F32)
            nc.sync.dma_start(out=srt, in_=bass.AP(srow_, 0, [[0, BQ], [1, C]]))

            # mtT[au, nk] = relu(Wm^T mk + bm); bias is per-partition here.
            # Matmuls are emitted chunk-major so the in-order PE queue tracks
            # the chunk DMA arrivals instead of serializing on the last one.
            mtT = pool.tile([128, 2, NK], BF16)
            attT = pool.tile([128, 2, BQ], BF16)
            groups = [(a, h) for a in range(2) for h in range(2)]
            pmts = []
            for _gi in range(len(groups)):
                pmt = psmt.tile([128, NK // 2], F32, tag="pmt")
                pmts.append(pmt)
            for c in range(4):
                for gi, (a, h) in enumerate(groups):
                    nc.tensor.matmul(
                        pmts[gi], wm[:, c, a * 128:(a + 1) * 128],
                        mkt[:, c, h * 512:(h + 1) * 512],
                        start=(c == 0), stop=(c == 3))
            for gi, (a, h) in enumerate(groups):
                # relu+bias alternates Act/DVE so four groups don't
                # serialize on one engine
                if gi % 2 == 0:
                    nc.scalar.activation(
                        out=mtT[:, a, h * 512:(h + 1) * 512], in_=pmts[gi],
                        func=RELU, bias=cst[:, 2 + a:3 + a], scale=1.0)
                else:
                    with nc.allow_low_precision(
                            reason="bf16 mt rounding ~0.4% on 2e-2 tol"):
                        nc.vector.tensor_scalar(
                            out=mtT[:, a, h * 512:(h + 1) * 512],
                            in0=pmts[gi], scalar1=cst[:, 2 + a:3 + a],
                            scalar2=0.0, op0=mybir.AluOpType.add,
                            op1=mybir.AluOpType.max)
            for gi, (a, h) in enumerate(groups):
                # attT[au, b] = sum_j mtT[au, (b j)] on DVE (fp32 internal)
                with nc.allow_low_precision(
                        reason="DVE reduces in fp32 internally; bf16 "
                               "output rounding is ~0.4% on 2e-2 tol"):
                    nc.vector.tensor_reduce(
                        out=attT[:, a, h * 16:(h + 1) * 16],
                        in_=mtT[:, a, h * 512:(h + 1) * 512].rearrange(
                            "p (b j) -> p b j", j=K),
                        axis=mybir.AxisListType.X, op=mybir.AluOpType.add)

            # qtT[au, b] = relu(Wq^T q + bq), fp32 matmuls (no ldweights)
            xT = pool.tile([128, 2, BQ], F32)
            for a in range(2):
                pqt = psq.tile([128, BQ], F32, tag="pqt")
                for c in range(4):
                    nc.tensor.matmul(
                        pqt, blob[:, BLOB_WQ + c * AU + a * 128:
                                  BLOB_WQ + c * AU + (a + 1) * 128],
                        qta[:, c, :], start=(c == 0), stop=(c == 3))
                qts = pool.tile([128, BQ], F32, tag=f"qts{a}")
                nc.scalar.activation(
                    out=qts, in_=pqt, func=RELU,
                    bias=cst[:, a:a + 1], scale=1.0)
                nc.vector.tensor_add(out=xT[:, a, :], in0=attT[:, a, :],
                                     in1=qts)

            # transpose xT -> x [b, au] (fp32)
            idt = blob[:, BLOB_ID:BLOB_ID + 128]
            x = pool.tile([BQ, AU], F32)
            st6 = pool.tile([BQ, 2, 6], F32)
            for a in range(2):
                pst = ps1.tile([BQ, 128], F32, tag="pst")
                nc.tensor.transpose(pst, xT[:, a, :], idt[:128, :128])
                nc.scalar.copy(out=x[:, a * 128:(a + 1) * 128], in_=pst)
                # layernorm stats per half, overlapping the other transpose;
                # gamma/beta and the (x-mu)*rstd affine are folded into Wc
                nc.vector.bn_stats(out=st6[:, a, :],
                                   in_=x[:, a * 128:(a + 1) * 128])
            st2 = pool.tile([BQ, 2], F32)
            nc.vector.bn_aggr(out=st2, in_=st6)
            eps = pool.tile([BQ, 1], F32)
            nc.vector.memset(eps, EPS_LN)
            sd = pool.tile([BQ, 1], F32)
            nc.scalar.activation(out=sd, in_=st2[:, 1:2],
                                 func=mybir.ActivationFunctionType.Sqrt,
                                 bias=eps, scale=1.0)
            rstd = pool.tile([BQ, 1], F32)
            nc.vector.reciprocal(out=rstd, in_=sd)

            # out = q @ Wc_q + bc' + rstd*(x @ Wc_ma' - mu*S); the x @ Wc_ma
            # product reads xT directly (x's transpose IS xT — the baseline's
            # round-trip transpose back to [au, b] was redundant)
            poq = ps1.tile([BQ, C], F32, tag="poq")
            for c in range(DC5):
                nc.tensor.matmul(
                    poq, qta[:, c, :],
                    blob[:, BLOB_WC + c * C:BLOB_WC + (c + 1) * C],
                    start=(c == 0), stop=(c == DC5 - 1))
            pom = ps1.tile([BQ, C], F32, tag="pom")
            for a in range(2):
                nc.tensor.matmul(
                    pom, xT[:, a, :],
                    blob[:, BLOB_WC + (5 + a) * C:BLOB_WC + (6 + a) * C],
                    start=(a == 0), stop=(a == 1))
            # out = poq + rstd*(pom - mu*S): f = mu*S runs early (only needs
            # mu), the rstd dependency lands in the final fused op
            f1 = pool.tile([BQ, C], F32)
            nc.vector.tensor_scalar(out=f1, in0=srt, scalar1=st2[:, 0:1],
                                    scalar2=None, op0=mybir.AluOpType.mult)
            e2 = pool.tile([BQ, C], F32)
            nc.vector.tensor_sub(out=e2, in0=pom, in1=f1)
            ot = pool.tile([BQ, C], F32)
            nc.vector.scalar_tensor_tensor(out=ot, in0=e2, scalar=rstd,
                                           in1=poq, op0=mybir.AluOpType.mult,
                                           op1=mybir.AluOpType.add)
            nc.sync.dma_start(out=out[:, :], in_=ot)
    nc.finalize()
    return nc


# ---------------------------------------------------------------------------
# SPMD runner with a persistent jitted executable
# ---------------------------------------------------------------------------


class _SpmdRunner:
    def __init__(self, nc, n_cores=NCORES):
        import jax
        from jax.sharding import Mesh, PartitionSpec
        from concourse.bass2jax import (
            _bass_exec_p,
            install_neuronx_cc_hook,
            partition_id_tensor,
        )

        try:
            from jax.experimental.shard_map import shard_map
        except ImportError:
            from jax.shard_map import shard_map

        install_neuronx_cc_hook()
        self.jax = jax
        partition_name = (
            nc.partition_id_tensor.name if nc.partition_id_tensor else None
        )
        in_names, out_names, out_avals, zero_outs = [], [], [], []
        for alloc in nc.m.functions[0].allocations:
            if not isinstance(alloc, mybir.MemoryLocationSet):
                continue
            name = alloc.memorylocations[0].name
            if alloc.kind == "ExternalInput":
                if name != partition_name:
                    in_names.append(name)
            elif alloc.kind == "ExternalOutput":
                shape = tuple(alloc.tensor_shape)
                dtype = mybir.dt.np(alloc.dtype)
                out_names.append(name)
                out_avals.append(jax.core.ShapedArray(shape, dtype))
                zero_outs.append(np.zeros((n_cores * shape[0], *shape[1:]), dtype))
        self.in_names = list(in_names)
        self.out_names = out_names
        self.out_avals = out_avals
        self.zero_outs = zero_outs
        self.n_cores = n_cores
        n_params = len(in_names)
        n_outs = len(out_names)
        all_in = in_names + out_names + ([partition_name] if partition_name else [])

        def _body(*args):
            operands = list(args)
            if partition_name is not None:
                operands.append(partition_id_tensor())
            return tuple(
                _bass_exec_p.bind(
                    *operands,
                    out_avals=tuple(out_avals),
                    in_names=tuple(all_in),
                    out_names=tuple(out_names),
                    lowering_input_output_aliases=(),
                    sim_require_finite=True,
                    sim_require_nnan=True,
                    nc=nc,
                )
            )

        devices = jax.devices()[:n_cores]
        mesh = Mesh(np.asarray(devices), ("core",))
        in_specs = (PartitionSpec("core"),) * (n_params + n_outs)
        out_specs = (PartitionSpec("core"),) * n_outs
        self.sharded = jax.jit(
            shard_map(
                _body, mesh=mesh, in_specs=in_specs, out_specs=out_specs,
                check_rep=False,
            ),
            donate_argnums=tuple(range(n_params, n_params + n_outs)),
            keep_unused=True,
        )

    def __call__(self, concat_in):
        args = [concat_in[n] for n in self.in_names]
        zeros = [np.zeros_like(z) for z in self.zero_outs]
        out_arrs = self.sharded(*args, *zeros)
        res = []
        for c in range(self.n_cores):
            res.append({
                name: np.asarray(out_arrs[i]).reshape(
                    self.n_cores, *self.out_avals[i].shape
                )[c]
                for i, name in enumerate(self.out_names)
            })
        return res


def _rep(a):
    a = np.ascontiguousarray(a)
    return np.broadcast_to(a, (NCORES,) + a.shape).reshape(
        NCORES * a.shape[0], *a.shape[1:]
    )


# ---------------------------------------------------------------------------
# Host orchestration
# ---------------------------------------------------------------------------


def kernel(**inputs):
    qe = np.asarray(inputs["query_embedding"], dtype=np.float32)
    keys = np.asarray(inputs["memory_keys"], dtype=np.float32)
    Wq = np.asarray(inputs["Wq"], dtype=np.float32)
    bq = np.asarray(inputs["bq"], dtype=np.float32)
    Wm = np.asarray(inputs["Wm"], dtype=np.float32)
    bm = np.asarray(inputs["bm"], dtype=np.float32)
    gam = np.asarray(inputs["ln_gamma"], dtype=np.float32)
    bet = np.asarray(inputs["ln_beta"], dtype=np.float32)
    Wc = np.asarray(inputs["Wc"], dtype=np.float32)
    bc_ = np.asarray(inputs["bc"], dtype=np.float32)
    k = int(inputs["k"])
    assert k == K and qe.shape == (B, D) and keys.shape == (N, D)

    # ---- host prep: normalize keys, fp8 layouts ----
    mn = np.sqrt(np.einsum("nd,nd->n", keys, keys, dtype=np.float64)).astype(np.float32)
    kn = keys * (1.0 / mn)[:, None]                 # [N, D] fp32, for rescoring
    qr_full = np.maximum(qe, 0.0)                   # [B, D] fp32 relu'd queries

    k8 = (kn * KSCALE).astype(F8NP)                 # [N, D] fp8
    q8T = np.ascontiguousarray(qr_full.T).astype(F8NP)   # [D, B] fp8
    q8_dev = q8T.reshape(2, 2, 128, B).transpose(2, 0, 1, 3)  # [128,2,2,B]

    import jax
    from jax.sharding import Mesh, NamedSharding, PartitionSpec
    devices = jax.devices()[:NCORES]
    mesh = Mesh(np.asarray(devices), ("core",))
    csh = NamedSharding(mesh, PartitionSpec("core"))

    parts = []
    for c in range(NCORES):
        kT = np.zeros((D, SHP), F8NP)
        kT[:, :SH] = k8[c * SH:(c + 1) * SH].T
        shard = np.ascontiguousarray(
            kT.reshape(2, 2, 128, SHP).transpose(2, 0, 1, 3))
        parts.append(jax.device_put(shard, devices[c]))
    k8_dev = jax.make_array_from_single_device_arrays(
        (NCORES * 128, 2, 2, SHP), csh, parts)

    if "r1" not in _cache:
        _cache["r1"] = _SpmdRunner(_build_phase1())
    res1 = _cache["r1"]({"q8": _rep(np.ascontiguousarray(q8_dev)), "k8": k8_dev})

    # ---- host merge: top-TOPG groups by fp16 group-max, exact rescore ----
    vals = np.empty((B, NCORES, NGTOT), np.float32)
    for c in range(NCORES):
        vals[:, c, :] = res1[c]["t8"].reshape(2 * 128, NGTOT)

    members = _slot_members()                   # [NGTOT, G] shard columns

    fvals = vals.reshape(B, NCORES * NGTOT)
    top = np.argpartition(-fvals, TOPG - 1, axis=1)[:, :TOPG]   # [B, TOPG]
    tcore, tslot = top // NGTOT, top % NGTOT
    cols = members[tslot]                                       # [B, TOPG, G]
    valid = cols < SH
    grow_ = tcore[..., None] * SH + np.where(valid, cols, 0)     # [B, TOPG, G]

    cand_rows = grow_.reshape(B, TOPG * G)
    cand_valid = valid.reshape(B, TOPG * G)

    # exact rescore (fp32): sims = kn[rows] . qr  (chunked over queries)
    sims = np.full((B, TOPG * G), -np.inf, np.float32)
    CH = 32
    for lo in range(0, B, CH):
        hi = lo + CH
        kr = kn[cand_rows[lo:hi]]                                # [CH, T*G, D]
        sims[lo:hi] = np.einsum("qkd,qd->qk", kr, qr_full[lo:hi],
                                optimize=True)
    sims[~cand_valid] = -np.inf

    order = np.argpartition(-sims, K - 1, axis=1)[:, :K]
    top_idx = np.take_along_axis(cand_rows, order, axis=1)       # [B, K]

    # ---- phase 2 ----
    if "r2" not in _cache:
        _cache["r2"] = _SpmdRunner(_build_phase2())
    r2 = _cache["r2"]

    wm_a = Wm.reshape(4, 128, AU).transpose(1, 0, 2)   # [128, 4, 256] fp32

    Wc_ma = gam[:, None] * Wc[512:768]                 # [256, C] gamma-folded
    bc_eff = bc_ + bet @ Wc[512:768]                   # beta folded into bias
    srow = Wc_ma.sum(axis=0).astype(np.float32)        # [C]

    blob = np.zeros((128, BLOB_W), np.float32)
    blob[:, :1024] = Wq.reshape(4, 128, AU).transpose(1, 0, 2).reshape(128, 1024)
    wc_a = np.zeros((7, 128, C), np.float32)
    wc_a[:4] = Wc[:512].reshape(4, 128, C)
    wc_a[4, 0] = bc_eff
    wc_a[5:7] = Wc_ma.reshape(2, 128, C)
    blob[:, BLOB_WC:BLOB_WC + 700] = wc_a.transpose(1, 0, 2).reshape(128, 700)
    blob[:, BLOB_ID:BLOB_ID + 128] = np.eye(128, dtype=np.float32)
    cst = np.empty((128, 4), np.float32)
    cst[:, 0:2] = bq.reshape(2, 128).T
    cst[:, 2:4] = bm.reshape(2, 128).T

    wmk_cc = np.empty((NCORES, 128, 4, AU + NK), BF16NP)
    wmk_cc[:, :, :, :AU] = wm_a.astype(BF16NP)[None]
    qta_cc = np.empty((NCORES, 128, DC5, BQ), np.float32)
    for c in range(NCORES):
        flat = top_idx[c * BQ:(c + 1) * BQ].reshape(NK)
        wmk_cc[c, :, :, AU:] = keys[flat].T.reshape(4, 128, NK).transpose(
            1, 0, 2).astype(BF16NP)
        q_aug = np.zeros((DC5 * 128, BQ), np.float32)
        q_aug[:D] = qr_full[c * BQ:(c + 1) * BQ].T
        q_aug[D] = 1.0
        qta_cc[c] = q_aug.reshape(DC5, 128, BQ).transpose(1, 0, 2)

    res2 = r2({
        "wmk": wmk_cc.reshape(NCORES * 128, 4, AU + NK),
        "qta": qta_cc.reshape(NCORES * 128, DC5, BQ),
        "blob": _rep(blob), "srow": _rep(srow), "cst": _rep(cst),
    })

    out = np.concatenate([res2[c]["out"] for c in range(NCORES)], axis=0)
    return out.astype(np.float32)

